# revision 7
# baseline (speedup 1.0000x reference)
"""Trainium2 Bass kernel for an 8-layer LoRA attention model.

Model (per layer): Wq_m = Wqkv + Aqkv@Bqkv; qkv = h @ Wq_m; causal MHA
(16 heads, hd=64); Wp_m = Wproj + Aproj@Bproj; h = h + attn_out @ Wp_m.
x: [2, 1024, 1024] fp32.

Distribution (8 cores): DP2 x TP4. Cores 0-3 handle batch 0, cores 4-7
batch 1. Within a group each core owns 4 heads: a 256-column shard of
Wq/Wk/Wv (Megatron split) and a 256-OUTPUT-column shard of Wproj. Per
layer the group does two AllGathers (attention outputs, then proj
outputs); there is no reduce, so results are bit-identical across the
group and numerically equal to the unsharded computation.

Layout: the residual stream is kept transposed on-chip: hT [1024 d, 1024
tokens]. The qk GEMM produces qT/kT directly ([dims, tokens]); v is
produced untransposed ([tokens, dims]) so it can serve as the stationary
operand of the attention AV matmul. Attention uses the scoresT [s2, s1]
orientation; softmax row-sums come from a ones-column appended to v
(AV output row 64 = sum of exp), so no partition-dim reductions or
transposes are ever needed.

All matmuls run in float32r (fp32 storage, ~12-bit mantissa operand
rounding, full PE rate) accumulating in fp32 PSUM; inputs are
pre-rounded on the host, on-chip matmul operands are written as f32r by
their producing DVE/ACT op.
"""

import numpy as np

import concourse.bass as bass
import concourse.mybir as mybir
import concourse.tile as tile
from concourse import bacc
from concourse.bass_utils import run_bass_kernel_spmd

F32 = mybir.dt.float32
F32R = mybir.dt.float32r
EXP = mybir.ActivationFunctionType.Exp

L = 8          # layers
D = 1024       # model dim
S = 1024       # sequence (tokens per batch == tokens per core)
HD = 64        # head dim
HPC = 4        # heads per core
HL = HPC * HD  # local head dims (256)
TP = 4         # tensor-parallel group size
N_CORES = 8
GROUPS = [[0, 1, 2, 3], [4, 5, 6, 7]]
NEG = -1.0e30

KT = D // 128   # k tiles of the model dim (8)
NJ = 2          # token chunks (512 each)
CW = S // NJ    # chunk width (512)


def round_f32r(x: np.ndarray) -> np.ndarray:
    """Round fp32 to the fp32r grid (round-to-nearest-even at bit 12)."""
    u = np.ascontiguousarray(x, np.float32).view(np.uint32).astype(np.uint64)
    lsb = np.uint64(1) << np.uint64(12)
    bias = (lsb >> np.uint64(1)) - np.uint64(1) + ((u >> np.uint64(12)) & np.uint64(1))
    r = (u + bias) & ~(lsb - np.uint64(1))
    return r.astype(np.uint32).view(np.float32)


def build_program(n_layers: int = L):
    nc = bacc.Bacc("TRN2", target_bir_lowering=False, debug=False,
                   num_devices=N_CORES)

    # ---- per-core external inputs (host pre-sharded, f32r-rounded) ----
    xT = nc.dram_tensor("xT", [D, S], F32R, kind="ExternalInput")
    wqk_d = nc.dram_tensor("wqk", [n_layers, D, 2 * HL], F32R, kind="ExternalInput")
    wv_d = nc.dram_tensor("wv", [n_layers, D, HL], F32R, kind="ExternalInput")
    wp_d = nc.dram_tensor("wp", [n_layers, D, HL], F32R, kind="ExternalInput")
    aqkT_d = nc.dram_tensor("aqkT", [n_layers, 16, D], F32R, kind="ExternalInput")
    bqk_d = nc.dram_tensor("bqk", [n_layers, 16, 2 * HL], F32R, kind="ExternalInput")
    bv_d = nc.dram_tensor("bv", [n_layers, 16, HL], F32R, kind="ExternalInput")
    apT_d = nc.dram_tensor("apT", [n_layers, 16, D], F32R, kind="ExternalInput")
    bp_d = nc.dram_tensor("bp", [n_layers, 16, HL], F32R, kind="ExternalInput")
    ones_d = nc.dram_tensor("vones", [128, HPC, 1], F32R, kind="ExternalInput")
    outT = nc.dram_tensor("outT", [D, S], F32R, kind="ExternalOutput")

    with tile.TileContext(nc) as tc:
        with (
            tc.tile_pool(name="per", bufs=1) as per,          # persistent
            tc.tile_pool(name="wpool", bufs=2) as wpool,      # qk weights (prefetch)
            tc.tile_pool(name="wpool1", bufs=1) as wpool1,    # v/p weights
            tc.tile_pool(name="lora", bufs=1) as lora,
            tc.tile_pool(name="expp", bufs=4) as expp,
            tc.tile_pool(name="opool", bufs=2) as opool,      # o chunks + norm temps
            tc.tile_pool(name="gat", bufs=1) as gat,          # gathered o (per chunk)
            tc.tile_pool(name="pf", bufs=3) as pf,            # gathered proj (streamed)
            tc.tile_pool(name="prs", bufs=2) as prs,          # proj shard staging
            tc.tile_pool(name="gps", bufs=2, space="PSUM") as gps,
            tc.tile_pool(name="sps", bufs=2, space="PSUM") as sps,
            tc.tile_pool(name="avps", bufs=4, space="PSUM") as avps,
            tc.tile_pool(name="dram", bufs=2, space="DRAM") as dram,
        ):
            # ---- persistent state ----
            hT = [per.tile([128, S], F32R, tag=f"hT{k}", name=f"hT{k}") for k in range(KT)]
            for k in range(KT):
                nc.sync.dma_start(hT[k][:], xT[k * 128:(k + 1) * 128, :])

            # qkT [512 rows, S]: rows 0..255 = qT (4 heads x 64),
            # 256..511 = kT.  4 tiles of [128, S].
            qkT = [per.tile([128, S], F32R, tag=f"qkT{m}", name=f"qkT{m}") for m in range(4)]

            # v_loc per s2-tile: [128, 4*65]; per head 64 v-dims + ones col
            v_loc = [per.tile([128, HPC * (HD + 1)], F32R, tag=f"v{i}", name=f"v{i}")
                     for i in range(S // 128)]
            for i in range(S // 128):
                vv = v_loc[i][:].rearrange("p (h e) -> p h e", e=HD + 1)
                nc.sync.dma_start(vv[:, :, HD:HD + 1], ones_d[:])

            # causal bias: [128,128] lower-strict triangle = NEG, else 0
            # (scoresT block [s2 within tile, s1 within square]; mask s2>s1)
            maskb = per.tile([128, 128], F32, tag="maskb", name="maskb")
            nc.gpsimd.memset(maskb[:], 0.0)
            nc.gpsimd.affine_select(
                out=maskb[:], in_=maskb[:],
                compare_op=mybir.AluOpType.is_ge,
                fill=NEG, base=0,
                pattern=[[1, 128]], channel_multiplier=-1,
            )

            for l in range(n_layers):
                # ================= weights + LoRA merge =================
                wqk = [wpool.tile([128, 2 * HL], F32R, tag=f"wqk{k}", name=f"wqk{k}") for k in range(KT)]
                wv = [wpool1.tile([128, HL], F32R, tag=f"wv{k}", name=f"wv{k}") for k in range(KT)]
                wp = [wpool1.tile([128, HL], F32R, tag=f"wp{k}", name=f"wp{k}") for k in range(KT)]
                for k in range(KT):
                    sl = slice(k * 128, (k + 1) * 128)
                    nc.sync.dma_start(wqk[k][:], wqk_d[l, sl, :])
                    nc.sync.dma_start(wv[k][:], wv_d[l, sl, :])
                    nc.sync.dma_start(wp[k][:], wp_d[l, sl, :])
                aqkT = lora.tile([16, D], F32R, tag="aqkT")
                bqk = lora.tile([16, 2 * HL], F32R, tag="bqk")
                bv = lora.tile([16, HL], F32R, tag="bv")
                apT = lora.tile([16, D], F32R, tag="apT")
                bp = lora.tile([16, HL], F32R, tag="bp")
                nc.sync.dma_start(aqkT[:], aqkT_d[l])
                nc.sync.dma_start(bqk[:], bqk_d[l])
                nc.sync.dma_start(bv[:], bv_d[l])
                nc.sync.dma_start(apT[:], apT_d[l])
                nc.sync.dma_start(bp[:], bp_d[l])

                for k in range(KT):
                    sl = slice(k * 128, (k + 1) * 128)
                    d_ps = gps.tile([128, 2 * HL], F32, tag="mm")
                    nc.tensor.matmul(d_ps[:], aqkT[:, sl], bqk[:], start=True, stop=True)
                    nc.vector.tensor_add(wqk[k][:], wqk[k][:], d_ps[:])
                    d2 = gps.tile([128, HL], F32, tag="mm")
                    nc.tensor.matmul(d2[:], aqkT[:, sl], bv[:], start=True, stop=True)
                    nc.vector.tensor_add(wv[k][:], wv[k][:], d2[:])
                    d3 = gps.tile([128, HL], F32, tag="mm")
                    nc.tensor.matmul(d3[:], apT[:, sl], bp[:], start=True, stop=True)
                    nc.vector.tensor_add(wp[k][:], wp[k][:], d3[:])

                # ================= qkT GEMM =================
                # qkT[m rows, chunk] = wqk[:, m-slice].T @ hT[:, chunk]
                for m in range(4):
                    msl = slice(m * 128, (m + 1) * 128)
                    for j in range(NJ):
                        jsl = slice(j * CW, (j + 1) * CW)
                        ps = gps.tile([128, CW], F32, tag="mm")
                        for k in range(KT):
                            nc.tensor.matmul(ps[:], wqk[k][:, msl], hT[k][:, jsl],
                                             start=(k == 0), stop=(k == KT - 1))
                        nc.vector.tensor_copy(qkT[m][:, jsl], ps[:])

                # ================= v GEMM (untransposed) =================
                # v[i s2-tile, head dims] = hT[:, i-slice].T @ wv
                for i in range(S // 128):
                    isl = slice(i * 128, (i + 1) * 128)
                    ps = gps.tile([128, HL], F32, tag="mm")
                    for k in range(KT):
                        nc.tensor.matmul(ps[:], hT[k][:, isl], wv[k][:],
                                         start=(k == 0), stop=(k == KT - 1))
                    vv = v_loc[i][:].rearrange("p (h e) -> p h e", e=HD + 1)
                    nc.vector.tensor_copy(vv[:, :, 0:HD],
                                          ps[:].rearrange("p (h e) -> p h e", e=HD))

                # ================= attention =================
                o_shard = dram.tile([HL, S], F32R, tag="o_shard")
                for j in range(NJ):
                    jsl = slice(j * CW, (j + 1) * CW)
                    n_i = 4 * j + 4  # causal: s2-tiles 0 .. 4j+3
                    for pair in range(2):
                        av = [avps.tile([HD + 1, CW], F32, tag="av", name="av") for _ in range(2)]
                        for i in range(n_i):
                            sq = i - 4 * j  # >=0 on the diagonal band
                            # kept cols within chunk: [max(sq,0)*128, CW)
                            c0 = max(sq, 0) * 128
                            for h2 in range(2):
                                hsl = slice(64 * h2, 64 * h2 + 64)
                                ps_s = sps.tile([128, CW], F32, tag="sc")
                                nc.tensor.matmul(
                                    ps_s[:, c0:CW],
                                    qkT[2 + pair][hsl, i * 128:(i + 1) * 128],
                                    qkT[pair][hsl, j * CW + c0:(j + 1) * CW],
                                    start=True, stop=True,
                                    tile_position=(64 * h2, 0),
                                )
                                if sq >= 0:
                                    nc.vector.tensor_add(
                                        ps_s[:, c0:c0 + 128],
                                        ps_s[:, c0:c0 + 128],
                                        maskb[:],
                                    )
                                e = expp.tile([128, CW], F32R, tag="e")
                                nc.scalar.activation(out=e[:, c0:CW],
                                                     in_=ps_s[:, c0:CW],
                                                     func=EXP, scale=0.125)
                                h = 2 * pair + h2
                                nc.tensor.matmul(
                                    av[h2][:, c0:CW],
                                    v_loc[i][:, h * (HD + 1):(h + 1) * (HD + 1)],
                                    e[:, c0:CW],
                                    start=(i == 0), stop=(i == n_i - 1),
                                )
                        # normalize: row HD of av = sum(exp); o = av/sum
                        for h2 in range(2):
                            h = 2 * pair + h2
                            recip = opool.tile([1, CW], F32, tag="recip")
                            nc.vector.reciprocal(recip[:], av[h2][HD:HD + 1, :])
                            rbc = opool.tile([HD, CW], F32, tag="rbc")
                            nc.gpsimd.partition_broadcast(rbc[:], recip[:])
                            o_j = opool.tile([HD, CW], F32R, tag="o_j")
                            nc.vector.tensor_mul(o_j[:], av[h2][0:HD, :], rbc[:])
                            nc.sync.dma_start(o_shard[h * HD:(h + 1) * HD, jsl], o_j[:])

                # ================= AllGather o =================
                o_full_d = dram.tile([D, S], F32R, tag="o_full")
                nc.gpsimd.collective_compute(
                    "AllGather", mybir.AluOpType.bypass, replica_groups=GROUPS,
                    ins=[o_shard.opt()], outs=[o_full_d.opt()],
                )
                # ================= proj (output-column shard) =================
                p_shard = dram.tile([HL, S], F32R, tag="p_shard")
                pr = [prs.tile([128, S], F32R, tag=f"pr{m}", name=f"pr{m}")
                      for m in range(2)]
                for j in range(NJ):
                    jsl = slice(j * CW, (j + 1) * CW)
                    ofull = [gat.tile([128, CW], F32R, tag=f"of{k}", name=f"of{k}")
                             for k in range(KT)]
                    for k in range(KT):
                        nc.sync.dma_start(ofull[k][:],
                                          o_full_d[k * 128:(k + 1) * 128, jsl])
                    for m in range(2):
                        msl = slice(m * 128, (m + 1) * 128)
                        ps = gps.tile([128, CW], F32, tag="mm")
                        for k in range(KT):
                            nc.tensor.matmul(ps[:], wp[k][:, msl], ofull[k][:],
                                             start=(k == 0), stop=(k == KT - 1))
                        nc.vector.tensor_copy(pr[m][:, jsl], ps[:])
                for m in range(2):
                    nc.sync.dma_start(p_shard[m * 128:(m + 1) * 128, :], pr[m][:])

                # ================= AllGather proj; residual =================
                p_full_d = dram.tile([D, S], F32R, tag="p_full")
                nc.gpsimd.collective_compute(
                    "AllGather", mybir.AluOpType.bypass, replica_groups=GROUPS,
                    ins=[p_shard.opt()], outs=[p_full_d.opt()],
                )
                for k in range(KT):
                    pfk = pf.tile([128, S], F32R, tag="pf")
                    nc.sync.dma_start(pfk[:], p_full_d[k * 128:(k + 1) * 128, :])
                    nc.vector.tensor_add(hT[k][:], hT[k][:], pfk[:])
                    if l == n_layers - 1:
                        nc.sync.dma_start(outT[k * 128:(k + 1) * 128, :], hT[k][:])

    nc.compile()
    return nc


def make_in_maps(inputs: dict, n_layers: int = L):
    x = np.asarray(inputs["x"], np.float32)
    Wqkv = np.asarray(inputs["Wqkv"], np.float32)[:n_layers]
    Aqkv = np.asarray(inputs["Aqkv"], np.float32)[:n_layers]
    Bqkv = np.asarray(inputs["Bqkv"], np.float32)[:n_layers]
    Wproj = np.asarray(inputs["Wproj"], np.float32)[:n_layers]
    Aproj = np.asarray(inputs["Aproj"], np.float32)[:n_layers]
    Bproj = np.asarray(inputs["Bproj"], np.float32)[:n_layers]

    in_maps = []
    for c in range(N_CORES):
        b, t = c // TP, c % TP
        cs = slice(HL * t, HL * t + HL)  # this core's head-dim columns
        wqk = np.concatenate([Wqkv[:, :, cs], Wqkv[:, :, D + HL * t:D + HL * t + HL]],
                             axis=2)
        bqk = np.concatenate([Bqkv[:, :, cs], Bqkv[:, :, D + HL * t:D + HL * t + HL]],
                             axis=2)
        in_maps.append({
            "vones": np.ones((128, HPC, 1), np.float32),
            "xT": round_f32r(x[b].T),
            "wqk": round_f32r(wqk),
            "wv": round_f32r(Wqkv[:, :, 2 * D + HL * t:2 * D + HL * t + HL]),
            "wp": round_f32r(Wproj[:, :, cs]),
            "aqkT": round_f32r(Aqkv.transpose(0, 2, 1)),
            "bqk": round_f32r(bqk),
            "bv": round_f32r(Bqkv[:, :, 2 * D + HL * t:2 * D + HL * t + HL]),
            "apT": round_f32r(Aproj.transpose(0, 2, 1)),
            "bp": round_f32r(Bproj[:, :, cs]),
        })
    return in_maps


_NC_CACHE = {}


def kernel(**inputs) -> np.ndarray:
    n_layers = L
    if n_layers not in _NC_CACHE:
        _NC_CACHE[n_layers] = build_program(n_layers)
    nc = _NC_CACHE[n_layers]
    in_maps = make_in_maps(inputs, n_layers)
    res = run_bass_kernel_spmd(nc, in_maps, core_ids=list(range(N_CORES)))
    out0 = res.results[0]["outT"].T
    out1 = res.results[TP]["outT"].T
    return np.stack([out0, out1]).astype(np.float32)


if __name__ == "__main__":
    rng = np.random.default_rng(0)
    s = 0.02
    inputs = {
        "x": rng.standard_normal((2, S, D)).astype(np.float32),
        "Wqkv": (rng.standard_normal((L, D, 3 * D)) * s).astype(np.float32),
        "Aqkv": (rng.standard_normal((L, D, 16)) * s).astype(np.float32),
        "Bqkv": (rng.standard_normal((L, 16, 3 * D)) * s).astype(np.float32),
        "Wproj": (rng.standard_normal((L, D, D)) * s).astype(np.float32),
        "Aproj": (rng.standard_normal((L, D, 16)) * s).astype(np.float32),
        "Bproj": (rng.standard_normal((L, 16, D)) * s).astype(np.float32),
    }
    out = kernel(**inputs)
    print("kernel output:", out.shape, out.dtype, float(np.abs(out).max()))


# revision 15
# speedup vs baseline: 1.2451x; 1.2451x over previous
"""Trainium2 Bass kernel for an 8-layer LoRA attention model.

Model (per layer): Wq_m = Wqkv + Aqkv@Bqkv; qkv = h @ Wq_m; causal MHA
(16 heads, hd=64); Wp_m = Wproj + Aproj@Bproj; h = h + attn_out @ Wp_m.
x: [2, 1024, 1024] fp32.

Distribution (8 cores): DP2 x TP4. Cores 0-3 handle batch 0, cores 4-7
batch 1. Within a group each core owns 4 heads: a 256-column shard of
Wq/Wk/Wv (Megatron split) and a 256-OUTPUT-column shard of Wproj. Per
layer the group does two AllGathers (attention outputs, then proj
outputs); there is no reduce, so results are bit-identical across the
group and numerically equal to the unsharded computation.

Layout: the residual stream is kept transposed on-chip: hT [1024 d, 1024
tokens]. The qk GEMM produces qT/kT directly ([dims, tokens]); v is
produced untransposed ([tokens, dims]) so it can serve as the stationary
operand of the attention AV matmul. Attention uses the scoresT [s2, s1]
orientation; softmax row-sums come from a ones-column appended to v
(AV output row 64 = sum of exp), so no partition-dim reductions or
transposes are ever needed.

All matmuls run in float32r (fp32 storage, ~12-bit mantissa operand
rounding, full PE rate) accumulating in fp32 PSUM; inputs are
pre-rounded on the host, on-chip matmul operands are written as f32r by
their producing DVE/ACT op.
"""

import numpy as np

import concourse.bass as bass
import concourse.mybir as mybir
import concourse.tile as tile
from concourse import bacc
from concourse.bass_utils import run_bass_kernel_spmd

F32 = mybir.dt.float32
BF16 = mybir.dt.bfloat16
F32R = mybir.dt.float32r
EXP = mybir.ActivationFunctionType.Exp

L = 8          # layers
D = 1024       # model dim
S = 1024       # sequence (tokens per batch == tokens per core)
HD = 64        # head dim
HPC = 4        # heads per core
HL = HPC * HD  # local head dims (256)
TP = 4         # tensor-parallel group size
N_CORES = 8
GROUPS = [[0, 1, 2, 3], [4, 5, 6, 7]]
NEG = -1.0e30

KT = D // 128   # k tiles of the model dim (8)
NJ = 2          # token chunks (512 each)
CW = S // NJ    # chunk width (512)


def round_f32r(x: np.ndarray) -> np.ndarray:
    """Round fp32 to the fp32r grid (round-to-nearest-even at bit 12)."""
    u = np.ascontiguousarray(x, np.float32).view(np.uint32).astype(np.uint64)
    lsb = np.uint64(1) << np.uint64(12)
    bias = (lsb >> np.uint64(1)) - np.uint64(1) + ((u >> np.uint64(12)) & np.uint64(1))
    r = (u + bias) & ~(lsb - np.uint64(1))
    return r.astype(np.uint32).view(np.float32)


def build_program(n_layers: int = L, reps: int = 1, no_cc: bool = False,
                  num_devices: int = N_CORES, wire_bf16: bool = True):
    nc = bacc.Bacc("TRN2", target_bir_lowering=False, debug=False,
                   num_devices=num_devices)
    WIRE = BF16 if wire_bf16 else F32R

    # ---- per-core external inputs (host pre-sharded, f32r-rounded) ----
    xT = nc.dram_tensor("xT", [D, S], F32R, kind="ExternalInput")
    wqk_d = nc.dram_tensor("wqk", [n_layers, D, 2 * HL], F32R, kind="ExternalInput")
    wv_d = nc.dram_tensor("wv", [n_layers, D, HL], F32R, kind="ExternalInput")
    wp_d = nc.dram_tensor("wp", [n_layers, D, HL], F32R, kind="ExternalInput")
    aq_d = nc.dram_tensor("aq", [n_layers, D, 16], F32R, kind="ExternalInput")
    bqk_d = nc.dram_tensor("bqk", [n_layers, 16, 2 * HL], F32R, kind="ExternalInput")
    bv_d = nc.dram_tensor("bv", [n_layers, 16, HL], F32R, kind="ExternalInput")
    ap_d = nc.dram_tensor("ap", [n_layers, D, 16], F32R, kind="ExternalInput")
    bp_d = nc.dram_tensor("bp", [n_layers, 16, HL], F32R, kind="ExternalInput")
    ones_d = nc.dram_tensor("vones", [128, HPC, 1], F32R, kind="ExternalInput")
    outT = nc.dram_tensor("outT", [D, S], F32R, kind="ExternalOutput")

    with tile.TileContext(nc) as tc:
        with (
            tc.tile_pool(name="per", bufs=1) as per,          # persistent
            tc.tile_pool(name="wpool", bufs=2) as wpool,      # qk weights (prefetch)
            tc.tile_pool(name="wpool1", bufs=1) as wpool1,    # v/p weights
            tc.tile_pool(name="lora", bufs=1) as lora,
            tc.tile_pool(name="expp", bufs=3) as expp,
            tc.tile_pool(name="opool", bufs=2) as opool,      # o chunks + norm temps
            tc.tile_pool(name="gat", bufs=1) as gat,          # gathered o (per chunk)
            tc.tile_pool(name="pf", bufs=2) as pf,            # gathered proj (streamed)
            tc.tile_pool(name="prs", bufs=2) as prs,          # proj shard staging
            tc.tile_pool(name="qkv2", bufs=2) as qkv2,
            tc.tile_pool(name="gps", bufs=3, space="PSUM") as gps,
            tc.tile_pool(name="sps", bufs=2, space="PSUM") as sps,
            tc.tile_pool(name="avps", bufs=3, space="PSUM") as avps,
            tc.tile_pool(name="dram", bufs=2, space="DRAM") as dram,
        ):
            # ---- persistent state ----
            hT = [per.tile([128, S], F32R, tag=f"hT{k}", name=f"hT{k}") for k in range(KT)]

            ones_sb = per.tile([128, HPC, 1], F32R, tag="ones", name="ones")
            nc.sync.dma_start(ones_sb[:], ones_d[:])

            # causal bias: [128,128] lower-strict triangle = NEG, else 0
            # (scoresT block [s2 within tile, s1 within square]; mask s2>s1)
            maskb = per.tile([128, 128], F32, tag="maskb", name="maskb")
            nc.gpsimd.memset(maskb[:], 0.0)
            nc.gpsimd.affine_select(
                out=maskb[:], in_=maskb[:],
                compare_op=mybir.AluOpType.is_ge,
                fill=NEG, base=0,
                pattern=[[1, 128]], channel_multiplier=-1,
            )

            for rep in range(reps):
              for k in range(KT):
                  nc.sync.dma_start(hT[k][:], xT[k * 128:(k + 1) * 128, :])
              for l in range(n_layers):
                # qkT [512 rows, S]: rows 0..255 = qT (4 heads x 64),
                # 256..511 = kT.  4 tiles of [128, S].  Double-buffered so
                # next layer's GEMMs overlap this layer's attention reads.
                qkT = [qkv2.tile([128, S], F32R, tag=f"qkT{m}", name=f"qkT{m}")
                       for m in range(4)]
                # v_loc per s2-tile: [128, 4*65]; per head 64 v-dims + ones col
                v_loc = [qkv2.tile([128, HPC * (HD + 1)], F32R, tag=f"v{i}",
                                   name=f"v{i}") for i in range(S // 128)]
                for i in range(S // 128):
                    vv = v_loc[i][:].rearrange("p (h e) -> p h e", e=HD + 1)
                    nc.vector.tensor_copy(vv[:, :, HD:HD + 1], ones_sb[:])
                # ================= weights + LoRA merge =================
                wqk_all = wpool.tile([128, KT, 2 * HL], F32R, tag="wqk",
                                     name="wqk_all")
                wv_all = wpool1.tile([128, KT, HL], F32R, tag="wv", name="wv_all")
                wp_all = wpool1.tile([128, KT, HL], F32R, tag="wp", name="wp_all")
                nc.scalar.dma_start(
                    wqk_all[:], wqk_d[l].rearrange("(k p) n -> p k n", p=128))
                nc.scalar.dma_start(
                    wv_all[:], wv_d[l].rearrange("(k p) n -> p k n", p=128))
                nc.scalar.dma_start(
                    wp_all[:], wp_d[l].rearrange("(k p) n -> p k n", p=128))
                wqk = [wqk_all[:, k, :] for k in range(KT)]
                wv = [wv_all[:, k, :] for k in range(KT)]
                wp = [wp_all[:, k, :] for k in range(KT)]
                aq_all = lora.tile([128, KT, 16], F32R, tag="aq", name="aq_all")
                ap_all = lora.tile([128, KT, 16], F32R, tag="ap", name="ap_all")
                nc.scalar.dma_start(
                    aq_all[:], aq_d[l].rearrange("(k p) n -> p k n", p=128))
                nc.scalar.dma_start(
                    ap_all[:], ap_d[l].rearrange("(k p) n -> p k n", p=128))
                aq = [aq_all[:, k, :] for k in range(KT)]
                ap = [ap_all[:, k, :] for k in range(KT)]
                bqk = lora.tile([16, 2 * HL], F32R, tag="bqk", name="bqk")
                bv = lora.tile([16, HL], F32R, tag="bv", name="bv")
                bp = lora.tile([16, HL], F32R, tag="bp", name="bp")
                nc.scalar.dma_start(bqk[:], bqk_d[l])
                nc.scalar.dma_start(bv[:], bv_d[l])
                nc.scalar.dma_start(bp[:], bp_d[l])

                # u = Aqkv^T @ h  [16, S], per chunk (LoRA inner product)
                u_qk = []
                for j in range(NJ):
                    jsl = slice(j * CW, (j + 1) * CW)
                    ups = gps.tile([16, CW], F32, tag="mm", name="ups")
                    for k in range(KT):
                        nc.tensor.matmul(ups[:], aq[k], hT[k][:, jsl],
                                         start=(k == 0), stop=(k == KT - 1))
                    uj = lora.tile([16, CW], F32R, tag=f"uqk{j}", name=f"uqk{j}")
                    nc.scalar.copy(uj[:], ups[:])
                    u_qk.append(uj)

                # ================= qkT GEMM (LoRA folded in) =================
                # qkT[m rows, chunk] = wqk[:, m].T @ hT[:, j] + bqk[:, m].T @ u
                for m in range(4):
                    msl = slice(m * 128, (m + 1) * 128)
                    for j in range(NJ):
                        jsl = slice(j * CW, (j + 1) * CW)
                        ps = gps.tile([128, CW], F32, tag="mm")
                        for k in range(KT):
                            nc.tensor.matmul(ps[:], wqk[k][:, msl], hT[k][:, jsl],
                                             start=(k == 0), stop=False)
                        nc.tensor.matmul(ps[:], bqk[:, msl], u_qk[j][:],
                                         start=False, stop=True)
                        nc.scalar.copy(qkT[m][:, jsl], ps[:])

                # ================= v GEMM (untransposed, LoRA folded) ========
                # v[i s2-tile, head dims] = hT[:, i].T @ wv + u[:, i].T @ bv
                for i in range(S // 128):
                    isl = slice(i * 128, (i + 1) * 128)
                    ps = gps.tile([128, HL], F32, tag="mm")
                    for k in range(KT):
                        nc.tensor.matmul(ps[:], hT[k][:, isl], wv[k],
                                         start=(k == 0), stop=False)
                    nc.tensor.matmul(
                        ps[:], u_qk[i // 4][:, (i % 4) * 128:(i % 4 + 1) * 128],
                        bv[:], start=False, stop=True)
                    vv = v_loc[i][:].rearrange("p (h e) -> p h e", e=HD + 1)
                    nc.vector.tensor_copy(vv[:, :, 0:HD],
                                          ps[:].rearrange("p (h e) -> p h e", e=HD))

                # ================= attention =================
                o_shard = [dram.tile([HL, CW], WIRE, tag=f"o_shard{j}",
                                     name=f"o_shard{j}") for j in range(NJ)]
                for j in range(NJ):
                    jsl = slice(j * CW, (j + 1) * CW)
                    n_i = 4 * j + 4  # causal: s2-tiles 0 .. 4j+3
                    for pair in range(2):
                        av = [avps.tile([HD + 1, CW], F32, tag="av", name="av") for _ in range(2)]
                        for i in range(n_i):
                            sq = i - 4 * j  # >=0 on the diagonal band
                            # kept cols within chunk: [max(sq,0)*128, CW)
                            c0 = max(sq, 0) * 128
                            for h2 in range(2):
                                hsl = slice(64 * h2, 64 * h2 + 64)
                                ps_s = sps.tile([128, CW], F32, tag="sc")
                                nc.tensor.matmul(
                                    ps_s[:, c0:CW],
                                    qkT[2 + pair][hsl, i * 128:(i + 1) * 128],
                                    qkT[pair][hsl, j * CW + c0:(j + 1) * CW],
                                    start=True, stop=True,
                                    tile_position=(64 * h2, 0),
                                )
                                if sq >= 0:
                                    nc.vector.tensor_add(
                                        ps_s[:, c0:c0 + 128],
                                        ps_s[:, c0:c0 + 128],
                                        maskb[:],
                                    )
                                e = expp.tile([128, CW], F32R, tag="e")
                                nc.scalar.activation(out=e[:, c0:CW],
                                                     in_=ps_s[:, c0:CW],
                                                     func=EXP, scale=0.125)
                                h = 2 * pair + h2
                                nc.tensor.matmul(
                                    av[h2][:, c0:CW],
                                    v_loc[i][:, h * (HD + 1):(h + 1) * (HD + 1)],
                                    e[:, c0:CW],
                                    start=(i == 0), stop=(i == n_i - 1),
                                )
                        # normalize: row HD of av = sum(exp); o = av/sum
                        for h2 in range(2):
                            h = 2 * pair + h2
                            recip = opool.tile([1, CW], F32, tag="recip")
                            nc.vector.reciprocal(recip[:], av[h2][HD:HD + 1, :])
                            rbc = opool.tile([HD, CW], F32, tag="rbc")
                            nc.gpsimd.partition_broadcast(rbc[:], recip[:])
                            o_j = opool.tile([HD, CW], WIRE, tag="o_j")
                            nc.vector.tensor_mul(o_j[:], av[h2][0:HD, :], rbc[:])
                            nc.sync.dma_start(o_shard[j][h * HD:(h + 1) * HD, :],
                                              o_j[:])

                # ====== per chunk: AG o -> proj -> AG p -> residual ======
                for j in range(NJ):
                    jsl = slice(j * CW, (j + 1) * CW)
                    o_full_d = dram.tile([D, CW], WIRE, tag=f"o_full{j}",
                                         name=f"o_full{j}")
                    if no_cc:
                        for q in range(TP):
                            nc.sync.dma_start(o_full_d[q * HL:(q + 1) * HL, :],
                                              o_shard[j][:])
                    else:
                        nc.gpsimd.collective_compute(
                            "AllGather", mybir.AluOpType.bypass,
                            replica_groups=GROUPS,
                            ins=[o_shard[j].opt()], outs=[o_full_d.opt()],
                        )
                    ofull_all = gat.tile([128, KT, CW], F32R, tag="of",
                                         name="ofull_all")
                    if wire_bf16:
                        ofraw = gat.tile([128, KT, CW], WIRE, tag="ofr",
                                         name="ofraw")
                        nc.sync.dma_start(
                            ofraw[:],
                            o_full_d[:].rearrange("(k p) n -> p k n", p=128))
                        nc.vector.tensor_copy(ofull_all[:], ofraw[:])
                    else:
                        nc.sync.dma_start(
                            ofull_all[:],
                            o_full_d[:].rearrange("(k p) n -> p k n", p=128))
                    ofull = [ofull_all[:, k, :] for k in range(KT)]
                    # u_p = Aproj^T @ o_full  [16, CW]
                    ups = gps.tile([16, CW], F32, tag="mm", name="upps")
                    for k in range(KT):
                        nc.tensor.matmul(ups[:], ap[k], ofull[k][:],
                                         start=(k == 0), stop=(k == KT - 1))
                    upj = lora.tile([16, CW], F32R, tag=f"up{j}", name=f"up{j}")
                    nc.scalar.copy(upj[:], ups[:])

                    p_shard = dram.tile([HL, CW], WIRE, tag=f"p_shard{j}",
                                        name=f"p_shard{j}")
                    for m in range(2):
                        msl = slice(m * 128, (m + 1) * 128)
                        ps = gps.tile([128, CW], F32, tag="mm")
                        for k in range(KT):
                            nc.tensor.matmul(ps[:], wp[k][:, msl], ofull[k],
                                             start=(k == 0), stop=False)
                        nc.tensor.matmul(ps[:], bp[:, msl], upj[:],
                                         start=False, stop=True)
                        prm = prs.tile([128, CW], WIRE, tag="pr", name="pr")
                        nc.scalar.copy(prm[:], ps[:])
                        nc.sync.dma_start(p_shard[m * 128:(m + 1) * 128, :],
                                          prm[:])

                    p_full_d = dram.tile([D, CW], WIRE, tag=f"p_full{j}",
                                         name=f"p_full{j}")
                    if no_cc:
                        for q in range(TP):
                            nc.sync.dma_start(p_full_d[q * HL:(q + 1) * HL, :],
                                              p_shard[:])
                    else:
                        nc.gpsimd.collective_compute(
                            "AllGather", mybir.AluOpType.bypass,
                            replica_groups=GROUPS,
                            ins=[p_shard.opt()], outs=[p_full_d.opt()],
                        )
                    pf_all = pf.tile([128, KT, CW], WIRE, tag="pf", name="pf_all")
                    nc.sync.dma_start(
                        pf_all[:], p_full_d[:].rearrange("(k p) n -> p k n", p=128))
                    for k in range(KT):
                        nc.vector.tensor_add(hT[k][:, jsl], hT[k][:, jsl],
                                             pf_all[:, k, :])
                        if l == n_layers - 1:
                            nc.sync.dma_start(outT[k * 128:(k + 1) * 128, jsl],
                                              hT[k][:, jsl])

    nc.compile()
    return nc


def make_in_maps(inputs: dict, n_layers: int = L):
    x = np.asarray(inputs["x"], np.float32)
    Wqkv = np.asarray(inputs["Wqkv"], np.float32)[:n_layers]
    Aqkv = np.asarray(inputs["Aqkv"], np.float32)[:n_layers]
    Bqkv = np.asarray(inputs["Bqkv"], np.float32)[:n_layers]
    Wproj = np.asarray(inputs["Wproj"], np.float32)[:n_layers]
    Aproj = np.asarray(inputs["Aproj"], np.float32)[:n_layers]
    Bproj = np.asarray(inputs["Bproj"], np.float32)[:n_layers]

    in_maps = []
    for c in range(N_CORES):
        b, t = c // TP, c % TP
        cs = slice(HL * t, HL * t + HL)  # this core's head-dim columns
        wqk = np.concatenate([Wqkv[:, :, cs], Wqkv[:, :, D + HL * t:D + HL * t + HL]],
                             axis=2)
        bqk = np.concatenate([Bqkv[:, :, cs], Bqkv[:, :, D + HL * t:D + HL * t + HL]],
                             axis=2)
        in_maps.append({
            "vones": np.ones((128, HPC, 1), np.float32),
            "xT": round_f32r(x[b].T),
            "wqk": round_f32r(wqk),
            "wv": round_f32r(Wqkv[:, :, 2 * D + HL * t:2 * D + HL * t + HL]),
            "wp": round_f32r(Wproj[:, :, cs]),
            "aq": round_f32r(Aqkv),
            "bqk": round_f32r(bqk),
            "bv": round_f32r(Bqkv[:, :, 2 * D + HL * t:2 * D + HL * t + HL]),
            "ap": round_f32r(Aproj),
            "bp": round_f32r(Bproj[:, :, cs]),
        })
    return in_maps


_NC_CACHE = {}


def kernel(**inputs) -> np.ndarray:
    n_layers = L
    if n_layers not in _NC_CACHE:
        _NC_CACHE[n_layers] = build_program(n_layers)
    nc = _NC_CACHE[n_layers]
    in_maps = make_in_maps(inputs, n_layers)
    res = run_bass_kernel_spmd(nc, in_maps, core_ids=list(range(N_CORES)))
    out0 = res.results[0]["outT"].T
    out1 = res.results[TP]["outT"].T
    return np.stack([out0, out1]).astype(np.float32)


if __name__ == "__main__":
    rng = np.random.default_rng(0)
    s = 0.02
    inputs = {
        "x": rng.standard_normal((2, S, D)).astype(np.float32),
        "Wqkv": (rng.standard_normal((L, D, 3 * D)) * s).astype(np.float32),
        "Aqkv": (rng.standard_normal((L, D, 16)) * s).astype(np.float32),
        "Bqkv": (rng.standard_normal((L, 16, 3 * D)) * s).astype(np.float32),
        "Wproj": (rng.standard_normal((L, D, D)) * s).astype(np.float32),
        "Aproj": (rng.standard_normal((L, D, 16)) * s).astype(np.float32),
        "Bproj": (rng.standard_normal((L, 16, D)) * s).astype(np.float32),
    }
    out = kernel(**inputs)
    print("kernel output:", out.shape, out.dtype, float(np.abs(out).max()))
